# revision 7
# baseline (speedup 1.0000x reference)
"""BEiT self-attention Trainium2 kernel (Bass/Tile), data-parallel over batch on 8 cores.

Layout strategy (per core, 8 batches):
  - hidden is pre-transposed on host to feature-major xT [768, 1576].
  - Q^T, K^T computed head-dim-major [o, m] (o = feature rows on partitions).
    The 1/sqrt(64) scale is folded into wq/bq on host.
  - V computed seq-major per batch with a ones column per head (65-wide head
    groups) so the probs@V matmul also yields the softmax row-sums.
  - Attention per (batch, head): scoresT[j, i] = k^T.T @ q^T with the i window
    padded to 256 columns (fp32r matmuls run 4x faster at free dim >= 256),
    bias add (DVE) + exp (ACT), ctx^T = [V | 1].T @ u, PE-transpose back to
    seq-major, normalize by reciprocal row-sum, DMA out.
"""

import numpy as np

import concourse.bacc as bacc
import concourse.mybir as mybir
from concourse.tile import TileContext
from concourse.bass_utils import run_bass_kernel_spmd

B, S, D, H, HD = 64, 197, 768, 12, 64
NCORES = 8
BPC = B // NCORES  # batches per core
F32 = mybir.dt.float32
F32R = mybir.dt.float32r
IW = 256  # padded query (i) window for attention matmuls
KT = D // 128  # 6 contraction tiles
OT = D // 128  # 6 output-feature tiles
JT = [(0, 128), (128, S - 128)]  # key/seq (j) partition tiles: 128 + 69
AluOp = mybir.AluOpType
ActFn = mybir.ActivationFunctionType




def build_program(bpc=BPC, group_sizes=None):
    if group_sizes is None:
        group_sizes = (4, 4) if bpc == 8 else (bpc,)
    assert sum(group_sizes) == bpc
    MTOT = bpc * S

    nc = bacc.Bacc("TRN2", target_bir_lowering=False, debug=False, num_devices=1)
    xT_d = nc.dram_tensor("xT", [D, MTOT], F32R, kind="ExternalInput")
    wqT_d = nc.dram_tensor("wqT", [D, D], F32R, kind="ExternalInput")
    wkT_d = nc.dram_tensor("wkT", [D, D], F32R, kind="ExternalInput")
    wvT_d = nc.dram_tensor("wvT", [D, D], F32R, kind="ExternalInput")
    bq_d = nc.dram_tensor("bq2", [128, OT], F32, kind="ExternalInput")
    bv_d = nc.dram_tensor("bvb", [128, D], F32, kind="ExternalInput")
    bt_d = nc.dram_tensor("biasT", [H, S, IW], F32, kind="ExternalInput")
    id_d = nc.dram_tensor("idm", [128, 128], F32, kind="ExternalInput")
    out_d = nc.dram_tensor("out", [MTOT, D], F32, kind="ExternalOutput")

    with TileContext(nc) as tc:
        with (
            tc.tile_pool(name="const", bufs=1) as cp,
            tc.tile_pool(name="grp", bufs=1) as gp,
            tc.tile_pool(name="work", bufs=3) as wp,
            tc.tile_pool(name="ps", bufs=2, space="PSUM") as pp,
        ):
            wq_t = [cp.tile([128, D], F32R, name=f"wq{k}", tag=f"wq{k}") for k in range(KT)]
            wk_t = [cp.tile([128, D], F32R, name=f"wk{k}", tag=f"wk{k}") for k in range(KT)]
            wv_t = [cp.tile([128, D], F32R, name=f"wv{k}", tag=f"wv{k}") for k in range(KT)]
            for k in range(KT):
                nc.sync.dma_start(wq_t[k][:], wqT_d[k * 128 : (k + 1) * 128, :])
                nc.sync.dma_start(wk_t[k][:], wkT_d[k * 128 : (k + 1) * 128, :])
                nc.sync.dma_start(wv_t[k][:], wvT_d[k * 128 : (k + 1) * 128, :])
            bqs = cp.tile([128, OT], F32, tag="bqs")
            nc.sync.dma_start(bqs[:], bq_d[:, :])
            bvb = cp.tile([128, D], F32, tag="bvb")
            nc.sync.dma_start(bvb[:], bv_d[:, :])
            idt = cp.tile([128, 128], F32, tag="idt")
            nc.sync.dma_start(idt[:], id_d[:, :])
            bias_t = {}
            for h in range(H):
                for jt, (j0, jw) in enumerate(JT):
                    t = cp.tile([128, IW], F32, name=f"bias{h}_{jt}", tag=f"bias{h}_{jt}")
                    nc.sync.dma_start(t[:jw, :], bt_d[h, j0 : j0 + jw, :])
                    bias_t[h, jt] = t

            b0 = 0
            for GB in group_sizes:
                MG = GB * S
                QW = MG + (IW - S)  # q tiles padded so the last batch's
                m0 = b0 * S  #         256-wide window stays in bounds
                xt = [gp.tile([128, MG], F32R, name=f"xt{k}", tag=f"xt{k}") for k in range(KT)]
                for k in range(KT):
                    nc.sync.dma_start(
                        xt[k][:], xT_d[k * 128 : (k + 1) * 128, m0 : m0 + MG]
                    )
                mchunks = []
                c = 0
                while c < MG:
                    w = min(512, MG - c)
                    mchunks.append((c, w))
                    c += w

                qt = [gp.tile([128, QW], F32R, name=f"qt{o}", tag=f"qt{o}") for o in range(OT)]
                kt = [gp.tile([128, MG], F32R, name=f"kt{o}", tag=f"kt{o}") for o in range(OT)]
                for o in range(OT):
                    for c0, cw in mchunks:
                        ps = pp.tile([128, 512], F32, name="pp", tag="pp")
                        for ki in range(KT):
                            nc.tensor.matmul(
                                ps[:, :cw],
                                (wq_t[ki][:, o * 128 : (o + 1) * 128]),
                                (xt[ki][:, c0 : c0 + cw]),
                                start=(ki == 0),
                                stop=(ki == KT - 1),
                            )
                        nc.scalar.activation(
                            qt[o][:, c0 : c0 + cw],
                            ps[:, :cw],
                            ActFn.Identity,
                            bias=bqs[:, o : o + 1],
                        )
                    nc.vector.memset(qt[o][:, MG:QW].bitcast(F32), 0.0)
                for o in range(OT):
                    for c0, cw in mchunks:
                        ps = pp.tile([128, 512], F32, name="pp", tag="pp")
                        for ki in range(KT):
                            nc.tensor.matmul(
                                ps[:, :cw],
                                (wk_t[ki][:, o * 128 : (o + 1) * 128]),
                                (xt[ki][:, c0 : c0 + cw]),
                                start=(ki == 0),
                                stop=(ki == KT - 1),
                            )
                        nc.vector.tensor_copy(kt[o][:, c0 : c0 + cw], ps[:, :cw])

                vt = {}
                for b in range(GB):
                    for jt, (j0, jw) in enumerate(JT):
                        v = gp.tile([128, H * 65], F32R, name=f"v{b}_{jt}", tag=f"v{b}_{jt}")
                        for c0, cw, h0 in [(0, 512, 0), (512, 256, 8)]:
                            nh = cw // 64
                            ps = pp.tile([128, 512], F32, name="pp", tag="pp")
                            for ki in range(KT):
                                nc.tensor.matmul(
                                    ps[:jw, :cw],
                                    (xt[ki][:, b * S + j0 : b * S + j0 + jw]),
                                    (wv_t[ki][:, c0 : c0 + cw]),
                                    start=(ki == 0),
                                    stop=(ki == KT - 1),
                                )
                            dst = v[:jw, :].rearrange("p (h c) -> p h c", c=65)[
                                :, h0 : h0 + nh, 0:64
                            ]
                            src = ps[:jw, :cw].rearrange("p (h c) -> p h c", c=64)
                            bsl = bvb[:jw, c0 : c0 + cw].rearrange(
                                "p (h c) -> p h c", c=64
                            )
                            nc.vector.tensor_tensor(dst, src, bsl, AluOp.add)
                        ones = v[:jw, :].rearrange("p (h c) -> p h c", c=65)[:, :, 64:65]
                        nc.vector.memset(ones.bitcast(F32), 1.0)
                        vt[b, jt] = v

                for b in range(GB):
                    for h in range(H):
                        o, po = h // 2, (h % 2) * 64
                        st = pp.tile([128, 512], F32, name="st", tag="st")
                        for jt, (j0, jw) in enumerate(JT):
                            nc.tensor.matmul(
                                st[:jw, jt * IW : (jt + 1) * IW],
                                (kt[o][po : po + 64, b * S + j0 : b * S + j0 + jw]),
                                (qt[o][po : po + 64, b * S : b * S + IW]),
                                start=True,
                                stop=True,
                            )
                        us = []
                        for jt, (j0, jw) in enumerate(JT):
                            u = wp.tile([128, IW], F32R, name=f"u{jt}", tag=f"u{jt}")
                            nc.vector.tensor_tensor(
                                u[:jw, :],
                                st[:jw, jt * IW : (jt + 1) * IW],
                                bias_t[h, jt][:jw, :],
                                AluOp.add,
                            )
                            nc.scalar.activation(u[:jw, :], u[:jw, :], ActFn.Exp)
                            us.append(u)
                        ct = pp.tile([65, 256], F32, name="ct", tag="ct")
                        for jt, (j0, jw) in enumerate(JT):
                            nc.tensor.matmul(
                                ct[:, :],
                                (vt[b, jt][:jw, h * 65 : (h + 1) * 65]),
                                (us[jt][:jw, :]),
                                start=(jt == 0),
                                stop=(jt == 1),
                            )
                        cs = wp.tile([65, S], F32, name="cs", tag="cs")
                        nc.scalar.copy(cs[:, :], ct[:65, :S])
                        pt = pp.tile([128, 130], F32, name="pt", tag="pt")
                        nc.tensor.transpose(
                            (pt[:128, 0:65]), (cs[:, 0:128]), (idt[:65, :65])
                        )
                        nc.tensor.transpose(
                            (pt[: S - 128, 65:130]), (cs[:, 128:S]), (idt[:65, :65])
                        )
                        row0 = (b0 + b) * S
                        for ib, (i0, iw2) in enumerate(JT):
                            rt = wp.tile([128, 1], F32, name=f"rt{ib}", tag=f"rt{ib}")
                            nc.vector.reciprocal(
                                rt[:iw2, :], pt[:iw2, ib * 65 + 64 : ib * 65 + 65]
                            )
                            ob = wp.tile([128, 64], F32, name=f"ob{ib}", tag=f"ob{ib}")
                            nc.vector.tensor_scalar_mul(
                                ob[:iw2, :], pt[:iw2, ib * 65 : ib * 65 + 64], rt[:iw2, :]
                            )
                            nc.sync.dma_start(
                                out_d[row0 + i0 : row0 + i0 + iw2, h * 64 : (h + 1) * 64],
                                ob[:iw2, :],
                            )
                b0 += GB
    nc.compile()
    return nc


def prep_host_inputs(inputs, bpc=BPC, cores=NCORES):
    """Shared (per-core-identical) tensors + per-core xT shards."""
    hs = np.ascontiguousarray(np.asarray(inputs["hidden_states"], dtype=np.float32))
    wq = np.asarray(inputs["wq"], np.float32)
    bq = np.asarray(inputs["bq"], np.float32)
    wk = np.asarray(inputs["wk"], np.float32)
    wv = np.asarray(inputs["wv"], np.float32)
    bv = np.asarray(inputs["bv"], np.float32)
    bias_table = np.asarray(inputs["bias_table"], np.float32)
    rel_index = np.asarray(inputs["rel_index"])

    scale = np.float32(1.0 / np.sqrt(HD))
    common = {
        "wqT": np.ascontiguousarray(wq.T) * scale,
        "wkT": np.ascontiguousarray(wk.T),
        "wvT": np.ascontiguousarray(wv.T),
        "bq2": np.ascontiguousarray((bq * scale).reshape(OT, 128).T),
        "bvb": np.ascontiguousarray(np.broadcast_to(bv, (128, D))),
        "idm": np.eye(128, dtype=np.float32),
    }
    rb = bias_table[rel_index]  # [S, S, H]
    biasT = np.zeros((H, S, IW), np.float32)
    biasT[:, :, :S] = rb.transpose(2, 1, 0)  # [h, j, i]
    common["biasT"] = biasT

    in_maps = []
    for c in range(cores):
        xc = hs[c * bpc : (c + 1) * bpc].reshape(bpc * S, D)
        in_maps.append({"xT": np.ascontiguousarray(xc.T), **common})
    return in_maps


_prog_cache = {}


def get_program(bpc=BPC, group_sizes=None):
    key = (bpc, group_sizes)
    if key not in _prog_cache:
        _prog_cache[key] = build_program(bpc, group_sizes)
    return _prog_cache[key]


def kernel(**inputs):
    nc = get_program()
    in_maps = prep_host_inputs(inputs)
    res = run_bass_kernel_spmd(nc, in_maps, list(range(NCORES)))
    out = np.concatenate([res.results[c]["out"] for c in range(NCORES)], axis=0)
    return out.reshape(B, S, D)
